# revision 20
# baseline (speedup 1.0000x reference)
"""Trainium2 Bass kernel for nn_AttentionMixer (two-stage grouped attention mixer).

Strategy (per core, data-parallel over batch B=16 -> 2 batches/core):
  - activations kept feature-major ("X^T": [feature, token]) for projections,
    produced by PE transposes of the token-major input.
  - Q^T, K^T projections feature-major (weights stationary); V projection
    token-major (activation tiles stationary) with an interleaved ones column
    (V_aug) so the AV matmul also produces the softmax denominator.
  - scores computed TRANSPOSED (lhsT=K^T, rhs=Q^T -> [s, l]) so that
    P^T = exp(scale*scores^T) is directly the AV stationary operand and the
    softmax normalizer lands on the PSUM partition axis (cheap per-partition
    tensor_scalar multiply at eviction). No max-subtraction (scores are small,
    |scaled| < ~1.4 -- validated against the reference).
  - all linear-layer V biases are folded host-side into downstream biases
    (softmax rows sum to 1 -> attention output shift = v_b), so only q/k
    biases (per-partition adds) and two broadcast bias tiles remain.
  - inter-stage token regroup (B*ng,(g*p)) -> (B*g,(ng*p)) is absorbed into
    the column access pattern of the stage-1 transpose evictions.
Everything in bf16 on the PE (fp32 PSUM accumulation).
"""

import numpy as np
import ml_dtypes

import concourse.bass as bass
import concourse.mybir as mybir
import concourse.tile as tile
from concourse import bacc
from concourse.masks import make_identity

BF16 = mybir.dt.bfloat16
F32 = mybir.dt.float32
AF = mybir.ActivationFunctionType

D = 512          # d_model
H = 8            # heads
E = 64           # head dim
L = 128          # tokens per attention sequence
NSEQ = 16        # sequences per stage per batch element
NT = 2048        # tokens per batch element
NKT = 4          # 512 // 128 contraction tiles
NB = 2           # batch elements per core
N_CORES = 8
SCALE = 0.125    # 1/sqrt(E)

W_NAMES = ["wq1", "wk1", "wv1", "wq2", "wk2", "wv2", "wo2"]


def _build_kernel():
    import os
    phase = os.environ.get("KPHASE", "full")
    nc = bacc.Bacc("TRN2", target_bir_lowering=False, debug=False)

    x_d = nc.dram_tensor("x", [NB * NT, D], F32, kind="ExternalInput")
    w_d = {n: nc.dram_tensor(n, [D, D], BF16, kind="ExternalInput") for n in W_NAMES}
    qb1_d = nc.dram_tensor("qb1", [128, NKT], F32, kind="ExternalInput")
    kb1_d = nc.dram_tensor("kb1", [128, NKT], F32, kind="ExternalInput")
    qb2_d = nc.dram_tensor("qb2", [128, NKT], F32, kind="ExternalInput")
    kb2_d = nc.dram_tensor("kb2", [128, NKT], F32, kind="ExternalInput")
    v2bc_d = nc.dram_tensor("v2bc", [128, D], F32, kind="ExternalInput")
    o2bc_d = nc.dram_tensor("o2bc", [128, D], F32, kind="ExternalInput")
    out_d = nc.dram_tensor("out", [NB * NT, D], F32, kind="ExternalOutput")

    with tile.TileContext(nc) as tc:
        with (
            tc.tile_pool(name="const", bufs=1) as const_pool,
            tc.tile_pool(name="big", bufs=1) as big,
            tc.tile_pool(name="work", bufs=3) as work,
            tc.tile_pool(name="psum", bufs=2, space="PSUM") as psum,
        ):
            # ---- constants ----
            ident32 = const_pool.tile([128, 128], F32, name="ident32", tag="ident32")
            make_identity(nc, ident32)
            ident16 = const_pool.tile([128, 128], BF16, name="ident16", tag="ident16")
            make_identity(nc, ident16)

            wsb = {}
            for n in W_NAMES:
                wsb[n] = const_pool.tile([128, NKT * D], BF16, name=f"sb_{n}", tag=f"sb_{n}")
                for ki in range(NKT):
                    nc.sync.dma_start(
                        out=wsb[n][:, ki * D:(ki + 1) * D],
                        in_=w_d[n][ki * 128:(ki + 1) * 128, :],
                    )
            biases = {}
            for n, dten in (("qb1", qb1_d), ("kb1", kb1_d), ("qb2", qb2_d), ("kb2", kb2_d)):
                biases[n] = const_pool.tile([128, NKT], F32, name=f"sb_{n}", tag=f"sb_{n}")
                nc.sync.dma_start(out=biases[n][:], in_=dten[:])
            v2bc = const_pool.tile([128, D], F32, name="sb_v2bc", tag="sb_v2bc")
            nc.sync.dma_start(out=v2bc[:], in_=v2bc_d[:])
            o2bc = const_pool.tile([128, D], F32, name="sb_o2bc", tag="sb_o2bc")
            nc.sync.dma_start(out=o2bc[:], in_=o2bc_d[:])

            for b in range(NB):
                # ---------- load + transpose x -> XT (feature-major bf16) ----------
                xt = big.tile([128, NKT * NT], BF16, name=f"xt_{b}", tag="xt")
                for tt in range(NSEQ):
                    xin = work.tile([128, D], F32, name=f"xin_{b}_{tt}", tag="xin")
                    nc.sync.dma_start(
                        out=xin[:], in_=x_d[b * NT + tt * 128: b * NT + (tt + 1) * 128, :]
                    )
                    for ki in range(NKT):
                        ps = psum.tile([128, 128], F32, name=f"pst_{b}_{tt}_{ki}", tag="tr")
                        nc.tensor.transpose(ps[:], xin[:, ki * 128:(ki + 1) * 128], ident32[:])
                        dst = xt[:, ki * NT + tt * 128: ki * NT + (tt + 1) * 128]
                        if ki % 2 == 0:
                            nc.vector.tensor_copy(dst, ps[:])
                        else:
                            nc.scalar.copy(dst, ps[:])

                def projections(src, wq, wk, wv, qb, kb, vbias_bc, qt, kt, vaug, pfx):
                    """src: [128, NKT*NT] bf16 feature-major.
                    qt/kt: feature-major outs (+per-partition bias).
                    vaug:  token-major V with interleaved ones cols (written strided);
                           vbias_bc: optional [128, D] broadcast bias tile."""
                    vview = vaug.rearrange("p (n h e) -> p n h e", n=NSEQ, h=H)
                    # ones columns (e == 64), one memset for the whole tensor
                    nc.vector.memset(vview[:, :, :, E], 1.0)
                    for tcn in range(NKT):  # 512-token chunks
                        tsl = slice(tcn * 512, (tcn + 1) * 512)
                        for o in range(NKT):
                            psq = psum.tile([128, 512], F32, name=f"{pfx}q_{tcn}_{o}", tag="proj")
                            for ki in range(NKT):
                                nc.tensor.matmul(
                                    psq[:],
                                    lhsT=wq[:, ki * D + o * 128: ki * D + (o + 1) * 128],
                                    rhs=src[:, ki * NT + tcn * 512: ki * NT + (tcn + 1) * 512],
                                    start=(ki == 0), stop=(ki == NKT - 1),
                                )
                            nc.vector.tensor_scalar_add(qt[:, o * NT:(o + 1) * NT][:, tsl], psq[:], qb[:, o:o + 1])
                            psk = psum.tile([128, 512], F32, name=f"{pfx}k_{tcn}_{o}", tag="proj")
                            for ki in range(NKT):
                                nc.tensor.matmul(
                                    psk[:],
                                    lhsT=wk[:, ki * D + o * 128: ki * D + (o + 1) * 128],
                                    rhs=src[:, ki * NT + tcn * 512: ki * NT + (tcn + 1) * 512],
                                    start=(ki == 0), stop=(ki == NKT - 1),
                                )
                            nc.scalar.add(kt[:, o * NT:(o + 1) * NT][:, tsl], psk[:], kb[:, o:o + 1])
                        for t4 in range(4):  # token-major V per 128-token tile
                            tt = tcn * 4 + t4
                            psv = psum.tile([128, 512], F32, name=f"{pfx}v_{tt}", tag="proj")
                            for ki in range(NKT):
                                nc.tensor.matmul(
                                    psv[:],
                                    lhsT=src[:, ki * NT + tt * 128: ki * NT + (tt + 1) * 128],
                                    rhs=wv[:, ki * D:(ki + 1) * D],
                                    start=(ki == 0), stop=(ki == NKT - 1),
                                )
                            dst = vview[:, tt, :, 0:E]  # [128, 8, 64] strided
                            psv_r = psv.rearrange("p (h e) -> p h e", h=H)
                            if vbias_bc is None:
                                nc.scalar.copy(dst, psv_r)
                            else:
                                nc.vector.tensor_add(
                                    dst, psv_r,
                                    vbias_bc.rearrange("p (h e) -> p h e", h=H),
                                )

                aphase = os.environ.get("KAPHASE", "tr")
                A_ORDER = ["sc", "exp", "av", "norm", "tr"]

                def attention(qt, kt, vaug, htgt, colmap, pfx):
                    """One attention stage over NSEQ sequences.
                    htgt: [128, NKT*NT] bf16 feature-major output.
                    colmap(seq, f_ki): destination column offset AP info for the
                    transpose eviction: returns (offset, ap_dims) applied to htgt."""
                    for s in range(NSEQ):
                        pssc = []
                        for bank in range(2):
                            p = psum.tile([128, 512], F32, name=f"{pfx}sc_{s}_{bank}", tag="sc")
                            pssc.append(p)
                        # heads are grouped by PE row-group per PSUM bank:
                        # bank = h % 2 so base-partition-0 and base-partition-64
                        # matmuls never share a bank (mixing them in one bank
                        # crashes the exec unit, NRT status 101).
                        for h in range(H):
                            bank, col = h % 2, h // 2
                            po = 64 * bank
                            fcol = (h // 2) * NT + s * 128
                            nc.tensor.matmul(
                                pssc[bank][:, col * 128:(col + 1) * 128],
                                lhsT=kt[po:po + 64, fcol:fcol + 128],
                                rhs=qt[po:po + 64, fcol:fcol + 128],
                                start=True, stop=True,
                            )
                        if A_ORDER.index(aphase) < 1:
                            continue
                        # pt column block for head h: (h%2)*512 + (h//2)*128
                        pt = work.tile([128, 1024], BF16, name=f"{pfx}pt_{s}", tag="pt", bufs=2)
                        nc.scalar.activation(pt[:, 0:512], pssc[0][:], AF.Exp, scale=SCALE)
                        nc.scalar.activation(pt[:, 512:1024], pssc[1][:], AF.Exp, scale=SCALE)
                        if A_ORDER.index(aphase) < 2:
                            continue
                        psav = []
                        for bank in range(2):
                            p = psum.tile([128, 4, E + 1], F32, name=f"{pfx}av_{s}_{bank}", tag="av")
                            psav.append(p)
                        for h in range(H):
                            bank, col = h % 2, h // 2
                            ptoff = bank * 512 + col * 128
                            nc.tensor.matmul(
                                psav[bank][:, col, :],
                                lhsT=pt[:, ptoff:ptoff + 128],
                                rhs=vaug[:, s * (H * (E + 1)) + h * (E + 1): s * (H * (E + 1)) + (h + 1) * (E + 1)],
                                start=True, stop=True,
                            )
                        if A_ORDER.index(aphase) < 3:
                            continue
                        htok = work.tile([128, D], BF16, name=f"{pfx}ht_{s}", tag="htok", bufs=2)
                        for bank in range(2):
                            rr = work.tile([128, 4], F32, name=f"{pfx}r_{s}_{bank}", tag="rr", bufs=2)
                            nc.vector.reciprocal(rr[:], psav[bank][:, :, E])
                            for col in range(4):
                                h = col * 2 + bank
                                if col % 2 == 0:
                                    nc.vector.tensor_scalar_mul(
                                        htok[:, h * E:(h + 1) * E],
                                        psav[bank][:, col, 0:E],
                                        rr[:, col:col + 1],
                                    )
                                else:
                                    nc.scalar.mul(
                                        htok[:, h * E:(h + 1) * E],
                                        psav[bank][:, col, 0:E],
                                        rr[:, col:col + 1],
                                    )
                        if A_ORDER.index(aphase) < 4:
                            continue
                        # transpose htok -> feature-major, scattered per colmap
                        for ki in range(NKT):
                            ps = psum.tile([128, 128], BF16, name=f"{pfx}tr_{s}_{ki}", tag="tr")
                            nc.tensor.transpose(ps[:], htok[:, ki * 128:(ki + 1) * 128], ident16[:])
                            dst = colmap(htgt, s, ki)
                            src = ps.rearrange("p (c w) -> p c w", c=NSEQ)
                            if ki % 2 == 0:
                                nc.vector.tensor_copy(dst, src)
                            else:
                                nc.scalar.copy(dst, src)

                # ---------- stage 1 ----------
                if phase == "xt":
                    continue
                qt1 = big.tile([128, NKT * NT], BF16, name=f"qt1_{b}", tag="qt")
                kt1 = big.tile([128, NKT * NT], BF16, name=f"kt1_{b}", tag="kt")
                va1 = big.tile([128, NSEQ * H * (E + 1)], BF16, name=f"va1_{b}", tag="vaug")
                projections(xt, wsb["wq1"], wsb["wk1"], wsb["wv1"],
                            biases["qb1"], biases["kb1"], None, qt1, kt1, va1, f"s1p{b}_")
                if phase == "s1p":
                    continue

                h1t = big.tile([128, NKT * NT], BF16, name=f"h1t_{b}", tag="h1t")

                def colmap1(tgt, s, ki):
                    # stage-1 seq s == group n; token (c, p) -> t2 = c*128 + n*8 + p
                    v = tgt.rearrange("q (k c w) -> q k c w", k=NKT, c=NSEQ)
                    return v[:, ki, :, s * 8:(s + 1) * 8]  # [128, 16, 8]

                attention(qt1, kt1, va1, h1t, colmap1, f"s1a{b}_")
                if phase == "s1a":
                    continue

                # ---------- stage 2 ----------
                qt2 = big.tile([128, NKT * NT], BF16, name=f"qt2_{b}", tag="qt")
                kt2 = big.tile([128, NKT * NT], BF16, name=f"kt2_{b}", tag="kt")
                va2 = big.tile([128, NSEQ * H * (E + 1)], BF16, name=f"va2_{b}", tag="vaug")
                projections(h1t, wsb["wq2"], wsb["wk2"], wsb["wv2"],
                            biases["qb2"], biases["kb2"], v2bc, qt2, kt2, va2, f"s2p{b}_")

                h2t = big.tile([128, NKT * NT], BF16, name=f"h2t_{b}", tag="h2t")

                def colmap2(tgt, s, ki):
                    sl = tgt[:, ki * NT + s * 128: ki * NT + (s + 1) * 128]
                    return sl.rearrange("p (c w) -> p c w", c=NSEQ)

                attention(qt2, kt2, va2, h2t, colmap2, f"s2a{b}_")
                if phase == "s2a":
                    continue

                # ---------- out2 projection (token-major) + store ----------
                out_v = out_d.rearrange("(bb n c p) d -> bb c n p d", bb=NB, n=NSEQ, c=NSEQ)
                for tt in range(NSEQ):  # stage-2 seq index c
                    pso = psum.tile([128, 512], F32, name=f"o2_{b}_{tt}", tag="proj")
                    for ki in range(NKT):
                        nc.tensor.matmul(
                            pso[:],
                            lhsT=h2t[:, ki * NT + tt * 128: ki * NT + (tt + 1) * 128],
                            rhs=wsb["wo2"][:, ki * D:(ki + 1) * D],
                            start=(ki == 0), stop=(ki == NKT - 1),
                        )
                    osb = work.tile([128, D], F32, name=f"osb_{b}_{tt}", tag="osb", bufs=2)
                    nc.vector.tensor_add(osb[:], pso[:], o2bc[:])
                    nc.sync.dma_start(out=out_v[b, tt], in_=osb[:])

    nc.compile()
    return nc


_NC_CACHE = {}


def _get_nc():
    if "nc" not in _NC_CACHE:
        _NC_CACHE["nc"] = _build_kernel()
    return _NC_CACHE["nc"]


def _prep_inputs(inputs):
    """Host-side data prep: shard x over cores, transpose+cast weights,
    fold V biases into downstream biases."""
    bf = ml_dtypes.bfloat16
    f32 = np.float32
    x = np.ascontiguousarray(np.asarray(inputs["x"], dtype=f32))  # [16,256,8,512]
    B = x.shape[0]
    xs = x.reshape(B, 256 * 8, D)

    g = {k: np.asarray(v, dtype=f32) for k, v in inputs.items() if k != "x"}
    q2_eb = g["q2_w"] @ g["v1_b"] + g["q2_b"]
    k2_eb = g["k2_w"] @ g["v1_b"] + g["k2_b"]
    v2_eb = g["v2_w"] @ g["v1_b"]
    o2_eb = g["out2_w"] @ g["v2_b"] + g["out2_b"]

    def wt(w):
        return np.ascontiguousarray(w.astype(bf).T)

    def btile(v):
        return np.ascontiguousarray(v.reshape(NKT, 128).T.astype(f32))

    common = {
        "wq1": wt(g["q1_w"]), "wk1": wt(g["k1_w"]), "wv1": wt(g["v1_w"]),
        "wq2": wt(g["q2_w"]), "wk2": wt(g["k2_w"]), "wv2": wt(g["v2_w"]),
        "wo2": wt(g["out2_w"]),
        "qb1": btile(g["q1_b"]), "kb1": btile(g["k1_b"]),
        "qb2": btile(q2_eb), "kb2": btile(k2_eb),
        "v2bc": np.ascontiguousarray(np.tile(v2_eb.astype(f32), (128, 1))),
        "o2bc": np.ascontiguousarray(np.tile(o2_eb.astype(f32), (128, 1))),
    }
    in_maps = []
    for c in range(N_CORES):
        m = dict(common)
        m["x"] = np.ascontiguousarray(
            xs[c * NB:(c + 1) * NB].reshape(NB * NT, D))
        in_maps.append(m)
    return in_maps


def _get_executor():
    """Build (once) a jitted shard_map executor over the 8 cores.

    Returns run(in_maps) -> list of per-core out arrays. Mirrors
    bass2jax.run_bass_via_pjrt but caches the jitted callable so repeat
    invocations don't retrace/recompile."""
    if "exec" in _NC_CACHE:
        return _NC_CACHE["exec"]

    import jax
    import concourse.mybir as mb
    from jax.sharding import Mesh, PartitionSpec
    from jax.experimental.shard_map import shard_map
    from concourse.bass2jax import (
        _bass_exec_p, install_neuronx_cc_hook, partition_id_tensor,
    )

    nc = _get_nc()
    install_neuronx_cc_hook()

    partition_name = nc.partition_id_tensor.name if nc.partition_id_tensor else None
    in_names = []
    out_names = []
    out_avals = []
    for alloc in nc.m.functions[0].allocations:
        if not isinstance(alloc, mb.MemoryLocationSet):
            continue
        name = alloc.memorylocations[0].name
        if alloc.kind == "ExternalInput":
            if name != partition_name:
                in_names.append(name)
        elif alloc.kind == "ExternalOutput":
            shape = tuple(alloc.tensor_shape)
            dtype = mb.dt.np(alloc.dtype)
            out_names.append(name)
            out_avals.append(jax.core.ShapedArray(shape, dtype))
    n_params = len(in_names)
    all_names = in_names + out_names
    if partition_name is not None:
        all_names = all_names + [partition_name]

    def _body(*args):
        operands = list(args)
        if partition_name is not None:
            operands.append(partition_id_tensor())
        outs = _bass_exec_p.bind(
            *operands,
            out_avals=tuple(out_avals),
            in_names=tuple(all_names),
            out_names=tuple(out_names),
            lowering_input_output_aliases=(),
            sim_require_finite=True,
            sim_require_nnan=True,
            nc=nc,
        )
        return tuple(outs)

    devices = jax.devices()[:N_CORES]
    mesh = Mesh(np.asarray(devices), ("core",))
    n_outs = len(out_names)
    sharded = jax.jit(
        shard_map(
            _body, mesh=mesh,
            in_specs=(PartitionSpec("core"),) * (n_params + n_outs),
            out_specs=(PartitionSpec("core"),) * n_outs,
            check_rep=False,
        ),
        keep_unused=True,
    )
    zero_outs = [np.zeros((N_CORES * a.shape[0], *a.shape[1:]), a.dtype)
                 for a in out_avals]

    def run(in_maps):
        concat_in = [
            np.concatenate([np.asarray(in_maps[c][nm]) for c in range(N_CORES)], axis=0)
            for nm in in_names
        ]
        out_arrs = sharded(*concat_in, *zero_outs)
        out = np.asarray(out_arrs[0])
        return [out.reshape(N_CORES, *out_avals[0].shape)[c] for c in range(N_CORES)]

    _NC_CACHE["exec"] = (run, sharded, in_names, zero_outs)
    return _NC_CACHE["exec"]


def run_kernel_results(inputs, trace=False):
    run, _, _, _ = _get_executor()
    in_maps = _prep_inputs(inputs)
    outs = run(in_maps)
    full = np.concatenate(
        [r.reshape(NB, 256, 8, D) for r in outs], axis=0).astype(np.float32)
    return full, None


def kernel(**inputs):
    full, _ = run_kernel_results(inputs)
    return full


# revision 23
# speedup vs baseline: 32.5790x; 32.5790x over previous
"""Trainium2 Bass kernel for nn_AttentionMixer (two-stage grouped attention mixer).

Strategy (per core, data-parallel over batch B=16 -> 2 batches/core):
  - activations kept feature-major ("X^T": [feature, token]) for projections,
    produced by PE transposes of the token-major input.
  - Q^T, K^T projections feature-major (weights stationary); V projection
    token-major (activation tiles stationary) with an interleaved ones column
    (V_aug) so the AV matmul also produces the softmax denominator.
  - scores computed TRANSPOSED (lhsT=K^T, rhs=Q^T -> [s, l]) so that
    P^T = exp(scale*scores^T) is directly the AV stationary operand and the
    softmax normalizer lands on the PSUM partition axis (cheap per-partition
    tensor_scalar multiply at eviction). No max-subtraction (scores are small,
    |scaled| < ~1.4 -- validated against the reference).
  - all linear-layer V biases are folded host-side into downstream biases
    (softmax rows sum to 1 -> attention output shift = v_b), so only q/k
    biases (per-partition adds) and two broadcast bias tiles remain.
  - inter-stage token regroup (B*ng,(g*p)) -> (B*g,(ng*p)) is absorbed into
    the column access pattern of the stage-1 transpose evictions.
Everything in bf16 on the PE (fp32 PSUM accumulation).
"""

import numpy as np
import ml_dtypes

import concourse.bass as bass
import concourse.mybir as mybir
import concourse.tile as tile
from concourse import bacc
from concourse.masks import make_identity

BF16 = mybir.dt.bfloat16
F32 = mybir.dt.float32
AF = mybir.ActivationFunctionType

D = 512          # d_model
H = 8            # heads
E = 64           # head dim
L = 128          # tokens per attention sequence
NSEQ = 16        # sequences per stage per batch element
NT = 2048        # tokens per batch element
NKT = 4          # 512 // 128 contraction tiles
NB = 2           # batch elements per core
N_CORES = 8
SCALE = 0.125    # 1/sqrt(E)

W_NAMES = ["wq1", "wk1", "wv1", "wq2", "wk2", "wv2", "wo2"]


def _build_kernel(repeat=1):
    import os
    phase = os.environ.get("KPHASE", "full")
    nc = bacc.Bacc("TRN2", target_bir_lowering=False, debug=False)

    x_d = nc.dram_tensor("x", [NB * NT, D], F32, kind="ExternalInput")
    w_d = {n: nc.dram_tensor(n, [D, D], BF16, kind="ExternalInput") for n in W_NAMES}
    qb1_d = nc.dram_tensor("qb1", [128, NKT], F32, kind="ExternalInput")
    kb1_d = nc.dram_tensor("kb1", [128, NKT], F32, kind="ExternalInput")
    qb2_d = nc.dram_tensor("qb2", [128, NKT], F32, kind="ExternalInput")
    kb2_d = nc.dram_tensor("kb2", [128, NKT], F32, kind="ExternalInput")
    v2bc_d = nc.dram_tensor("v2bc", [128, D], F32, kind="ExternalInput")
    o2bc_d = nc.dram_tensor("o2bc", [128, D], F32, kind="ExternalInput")
    out_d = nc.dram_tensor("out", [NB * NT, D], F32, kind="ExternalOutput")

    with tile.TileContext(nc) as tc:
        with (
            tc.tile_pool(name="const", bufs=1) as const_pool,
            tc.tile_pool(name="big", bufs=1) as big,
            tc.tile_pool(name="work", bufs=3) as work,
            tc.tile_pool(name="psum", bufs=2, space="PSUM") as psum,
        ):
            # ---- constants ----
            ident32 = const_pool.tile([128, 128], F32, name="ident32", tag="ident32")
            make_identity(nc, ident32)
            ident16 = const_pool.tile([128, 128], BF16, name="ident16", tag="ident16")
            make_identity(nc, ident16)

            wsb = {}
            for n in W_NAMES:
                wsb[n] = const_pool.tile([128, NKT * D], BF16, name=f"sb_{n}", tag=f"sb_{n}")
                for ki in range(NKT):
                    nc.sync.dma_start(
                        out=wsb[n][:, ki * D:(ki + 1) * D],
                        in_=w_d[n][ki * 128:(ki + 1) * 128, :],
                    )
            biases = {}
            for n, dten in (("qb1", qb1_d), ("kb1", kb1_d), ("qb2", qb2_d), ("kb2", kb2_d)):
                biases[n] = const_pool.tile([128, NKT], F32, name=f"sb_{n}", tag=f"sb_{n}")
                nc.sync.dma_start(out=biases[n][:], in_=dten[:])
            v2bc = const_pool.tile([128, D], F32, name="sb_v2bc", tag="sb_v2bc")
            nc.sync.dma_start(out=v2bc[:], in_=v2bc_d[:])
            o2bc = const_pool.tile([128, D], F32, name="sb_o2bc", tag="sb_o2bc")
            nc.sync.dma_start(out=o2bc[:], in_=o2bc_d[:])

            for b in [bb % NB for bb in range(NB * repeat)]:
                # ---------- load + transpose x -> XT (feature-major bf16) ----------
                xt = big.tile([128, NKT * NT], BF16, name=f"xt_{b}", tag="xt")
                for tt in range(NSEQ):
                    xin = work.tile([128, D], F32, name=f"xin_{b}_{tt}", tag="xin")
                    nc.sync.dma_start(
                        out=xin[:], in_=x_d[b * NT + tt * 128: b * NT + (tt + 1) * 128, :]
                    )
                    for ki in range(NKT):
                        ps = psum.tile([128, 128], F32, name=f"pst_{b}_{tt}_{ki}", tag="tr")
                        nc.tensor.transpose(ps[:], xin[:, ki * 128:(ki + 1) * 128], ident32[:])
                        dst = xt[:, ki * NT + tt * 128: ki * NT + (tt + 1) * 128]
                        if ki % 2 == 0:
                            nc.vector.tensor_copy(dst, ps[:])
                        else:
                            nc.scalar.copy(dst, ps[:])

                def projections(src, wq, wk, wv, qb, kb, vbias_bc, qt, kt, vaug, pfx):
                    """src: [128, NKT*NT] bf16 feature-major.
                    qt/kt: feature-major outs (+per-partition bias).
                    vaug:  token-major V with interleaved ones cols (written strided);
                           vbias_bc: optional [128, D] broadcast bias tile."""
                    vview = vaug.rearrange("p (n h e) -> p n h e", n=NSEQ, h=H)
                    # ones columns (e == 64), one memset for the whole tensor
                    nc.vector.memset(vview[:, :, :, E], 1.0)
                    for tcn in range(NKT):  # 512-token chunks
                        tsl = slice(tcn * 512, (tcn + 1) * 512)
                        for o in range(NKT):
                            psq = psum.tile([128, 512], F32, name=f"{pfx}q_{tcn}_{o}", tag="proj")
                            for ki in range(NKT):
                                nc.tensor.matmul(
                                    psq[:],
                                    lhsT=wq[:, ki * D + o * 128: ki * D + (o + 1) * 128],
                                    rhs=src[:, ki * NT + tcn * 512: ki * NT + (tcn + 1) * 512],
                                    start=(ki == 0), stop=(ki == NKT - 1),
                                )
                            nc.vector.tensor_scalar_add(qt[:, o * NT:(o + 1) * NT][:, tsl], psq[:], qb[:, o:o + 1])
                            psk = psum.tile([128, 512], F32, name=f"{pfx}k_{tcn}_{o}", tag="proj")
                            for ki in range(NKT):
                                nc.tensor.matmul(
                                    psk[:],
                                    lhsT=wk[:, ki * D + o * 128: ki * D + (o + 1) * 128],
                                    rhs=src[:, ki * NT + tcn * 512: ki * NT + (tcn + 1) * 512],
                                    start=(ki == 0), stop=(ki == NKT - 1),
                                )
                            nc.scalar.add(kt[:, o * NT:(o + 1) * NT][:, tsl], psk[:], kb[:, o:o + 1])
                        for t4 in range(4):  # token-major V per 128-token tile
                            tt = tcn * 4 + t4
                            psv = psum.tile([128, 512], F32, name=f"{pfx}v_{tt}", tag="proj")
                            for ki in range(NKT):
                                nc.tensor.matmul(
                                    psv[:],
                                    lhsT=src[:, ki * NT + tt * 128: ki * NT + (tt + 1) * 128],
                                    rhs=wv[:, ki * D:(ki + 1) * D],
                                    start=(ki == 0), stop=(ki == NKT - 1),
                                )
                            dst = vview[:, tt, :, 0:E]  # [128, 8, 64] strided
                            psv_r = psv.rearrange("p (h e) -> p h e", h=H)
                            if vbias_bc is None:
                                nc.scalar.copy(dst, psv_r)
                            else:
                                nc.vector.tensor_add(
                                    dst, psv_r,
                                    vbias_bc.rearrange("p (h e) -> p h e", h=H),
                                )

                aphase = os.environ.get("KAPHASE", "tr")
                A_ORDER = ["sc", "exp", "av", "norm", "tr"]

                def attention(qt, kt, vaug, htgt, colmap, pfx):
                    """One attention stage over NSEQ sequences.
                    htgt: [128, NKT*NT] bf16 feature-major output.
                    colmap(seq, f_ki): destination column offset AP info for the
                    transpose eviction: returns (offset, ap_dims) applied to htgt."""
                    for s in range(NSEQ):
                        pssc = []
                        for bank in range(2):
                            p = psum.tile([128, 512], F32, name=f"{pfx}sc_{s}_{bank}", tag="sc")
                            pssc.append(p)
                        # heads are grouped by PE row-group per PSUM bank:
                        # bank = h % 2 so base-partition-0 and base-partition-64
                        # matmuls never share a bank (mixing them in one bank
                        # crashes the exec unit, NRT status 101).
                        for h in range(H):
                            bank, col = h % 2, h // 2
                            po = 64 * bank
                            fcol = (h // 2) * NT + s * 128
                            nc.tensor.matmul(
                                pssc[bank][:, col * 128:(col + 1) * 128],
                                lhsT=kt[po:po + 64, fcol:fcol + 128],
                                rhs=qt[po:po + 64, fcol:fcol + 128],
                                start=True, stop=True,
                            )
                        if A_ORDER.index(aphase) < 1:
                            continue
                        # pt column block for head h: (h%2)*512 + (h//2)*128
                        pt = work.tile([128, 1024], BF16, name=f"{pfx}pt_{s}", tag="pt", bufs=2)
                        nc.scalar.activation(pt[:, 0:512], pssc[0][:], AF.Exp, scale=SCALE)
                        nc.scalar.activation(pt[:, 512:1024], pssc[1][:], AF.Exp, scale=SCALE)
                        if A_ORDER.index(aphase) < 2:
                            continue
                        psav = []
                        for bank in range(2):
                            p = psum.tile([128, 4, E + 1], F32, name=f"{pfx}av_{s}_{bank}", tag="av")
                            psav.append(p)
                        for h in range(H):
                            bank, col = h % 2, h // 2
                            ptoff = bank * 512 + col * 128
                            nc.tensor.matmul(
                                psav[bank][:, col, :],
                                lhsT=pt[:, ptoff:ptoff + 128],
                                rhs=vaug[:, s * (H * (E + 1)) + h * (E + 1): s * (H * (E + 1)) + (h + 1) * (E + 1)],
                                start=True, stop=True,
                            )
                        if A_ORDER.index(aphase) < 3:
                            continue
                        htok = work.tile([128, D], BF16, name=f"{pfx}ht_{s}", tag="htok", bufs=2)
                        for bank in range(2):
                            rr = work.tile([128, 4], F32, name=f"{pfx}r_{s}_{bank}", tag="rr", bufs=2)
                            nc.vector.reciprocal(rr[:], psav[bank][:, :, E])
                            for col in range(4):
                                h = col * 2 + bank
                                if col % 2 == 0:
                                    nc.vector.tensor_scalar_mul(
                                        htok[:, h * E:(h + 1) * E],
                                        psav[bank][:, col, 0:E],
                                        rr[:, col:col + 1],
                                    )
                                else:
                                    nc.scalar.mul(
                                        htok[:, h * E:(h + 1) * E],
                                        psav[bank][:, col, 0:E],
                                        rr[:, col:col + 1],
                                    )
                        if A_ORDER.index(aphase) < 4:
                            continue
                        # transpose htok -> feature-major, scattered per colmap
                        for ki in range(NKT):
                            ps = psum.tile([128, 128], BF16, name=f"{pfx}tr_{s}_{ki}", tag="tr")
                            nc.tensor.transpose(ps[:], htok[:, ki * 128:(ki + 1) * 128], ident16[:])
                            dst = colmap(htgt, s, ki)
                            src = ps.rearrange("p (c w) -> p c w", c=NSEQ)
                            if ki % 2 == 0:
                                nc.vector.tensor_copy(dst, src)
                            else:
                                nc.scalar.copy(dst, src)

                # ---------- stage 1 ----------
                if phase == "xt":
                    continue
                qt1 = big.tile([128, NKT * NT], BF16, name=f"qt1_{b}", tag="qt")
                kt1 = big.tile([128, NKT * NT], BF16, name=f"kt1_{b}", tag="kt")
                va1 = big.tile([128, NSEQ * H * (E + 1)], BF16, name=f"va1_{b}", tag="vaug")
                projections(xt, wsb["wq1"], wsb["wk1"], wsb["wv1"],
                            biases["qb1"], biases["kb1"], None, qt1, kt1, va1, f"s1p{b}_")
                if phase == "s1p":
                    continue

                h1t = big.tile([128, NKT * NT], BF16, name=f"h1t_{b}", tag="h1t")

                def colmap1(tgt, s, ki):
                    # stage-1 seq s == group n; token (c, p) -> t2 = c*128 + n*8 + p
                    v = tgt.rearrange("q (k c w) -> q k c w", k=NKT, c=NSEQ)
                    return v[:, ki, :, s * 8:(s + 1) * 8]  # [128, 16, 8]

                attention(qt1, kt1, va1, h1t, colmap1, f"s1a{b}_")
                if phase == "s1a":
                    continue

                # ---------- stage 2 ----------
                qt2 = big.tile([128, NKT * NT], BF16, name=f"qt2_{b}", tag="qt")
                kt2 = big.tile([128, NKT * NT], BF16, name=f"kt2_{b}", tag="kt")
                va2 = big.tile([128, NSEQ * H * (E + 1)], BF16, name=f"va2_{b}", tag="vaug")
                projections(h1t, wsb["wq2"], wsb["wk2"], wsb["wv2"],
                            biases["qb2"], biases["kb2"], v2bc, qt2, kt2, va2, f"s2p{b}_")

                h2t = big.tile([128, NKT * NT], BF16, name=f"h2t_{b}", tag="h2t")

                def colmap2(tgt, s, ki):
                    sl = tgt[:, ki * NT + s * 128: ki * NT + (s + 1) * 128]
                    return sl.rearrange("p (c w) -> p c w", c=NSEQ)

                attention(qt2, kt2, va2, h2t, colmap2, f"s2a{b}_")
                if phase == "s2a":
                    continue

                # ---------- out2 projection (token-major) + store ----------
                out_v = out_d.rearrange("(bb n c p) d -> bb c n p d", bb=NB, n=NSEQ, c=NSEQ)
                for tt in range(NSEQ):  # stage-2 seq index c
                    pso = psum.tile([128, 512], F32, name=f"o2_{b}_{tt}", tag="proj")
                    for ki in range(NKT):
                        nc.tensor.matmul(
                            pso[:],
                            lhsT=h2t[:, ki * NT + tt * 128: ki * NT + (tt + 1) * 128],
                            rhs=wsb["wo2"][:, ki * D:(ki + 1) * D],
                            start=(ki == 0), stop=(ki == NKT - 1),
                        )
                    osb = work.tile([128, D], F32, name=f"osb_{b}_{tt}", tag="osb", bufs=2)
                    nc.vector.tensor_add(osb[:], pso[:], o2bc[:])
                    nc.sync.dma_start(out=out_v[b, tt], in_=osb[:])

    nc.compile()
    return nc


_NC_CACHE = {}


def _get_nc(repeat=1):
    key = ("nc", repeat)
    if key not in _NC_CACHE:
        _NC_CACHE[key] = _build_kernel(repeat)
    return _NC_CACHE[key]


def _prep_inputs(inputs):
    """Host-side data prep: shard x over cores, transpose+cast weights,
    fold V biases into downstream biases."""
    bf = ml_dtypes.bfloat16
    f32 = np.float32
    x = np.ascontiguousarray(np.asarray(inputs["x"], dtype=f32))  # [16,256,8,512]
    B = x.shape[0]
    xs = x.reshape(B, 256 * 8, D)

    g = {k: np.asarray(v, dtype=f32) for k, v in inputs.items() if k != "x"}
    q2_eb = g["q2_w"] @ g["v1_b"] + g["q2_b"]
    k2_eb = g["k2_w"] @ g["v1_b"] + g["k2_b"]
    v2_eb = g["v2_w"] @ g["v1_b"]
    o2_eb = g["out2_w"] @ g["v2_b"] + g["out2_b"]

    def wt(w):
        return np.ascontiguousarray(w.astype(bf).T)

    def btile(v):
        return np.ascontiguousarray(v.reshape(NKT, 128).T.astype(f32))

    common = {
        "wq1": wt(g["q1_w"]), "wk1": wt(g["k1_w"]), "wv1": wt(g["v1_w"]),
        "wq2": wt(g["q2_w"]), "wk2": wt(g["k2_w"]), "wv2": wt(g["v2_w"]),
        "wo2": wt(g["out2_w"]),
        "qb1": btile(g["q1_b"]), "kb1": btile(g["k1_b"]),
        "qb2": btile(q2_eb), "kb2": btile(k2_eb),
        "v2bc": np.ascontiguousarray(np.tile(v2_eb.astype(f32), (128, 1))),
        "o2bc": np.ascontiguousarray(np.tile(o2_eb.astype(f32), (128, 1))),
    }
    in_maps = []
    for c in range(N_CORES):
        m = dict(common)
        m["x"] = np.ascontiguousarray(
            xs[c * NB:(c + 1) * NB].reshape(NB * NT, D))
        in_maps.append(m)
    return in_maps


def _get_executor(repeat=1):
    """Build (once) a jitted shard_map executor over the 8 cores.

    Returns run(in_maps) -> list of per-core out arrays. Mirrors
    bass2jax.run_bass_via_pjrt but caches the jitted callable so repeat
    invocations don't retrace/recompile."""
    key = ("exec", repeat)
    if key in _NC_CACHE:
        return _NC_CACHE[key]

    import jax
    import concourse.mybir as mb
    from jax.sharding import Mesh, PartitionSpec
    from jax.experimental.shard_map import shard_map
    from concourse.bass2jax import (
        _bass_exec_p, install_neuronx_cc_hook, partition_id_tensor,
    )

    nc = _get_nc(repeat)
    install_neuronx_cc_hook()

    partition_name = nc.partition_id_tensor.name if nc.partition_id_tensor else None
    in_names = []
    out_names = []
    out_avals = []
    for alloc in nc.m.functions[0].allocations:
        if not isinstance(alloc, mb.MemoryLocationSet):
            continue
        name = alloc.memorylocations[0].name
        if alloc.kind == "ExternalInput":
            if name != partition_name:
                in_names.append(name)
        elif alloc.kind == "ExternalOutput":
            shape = tuple(alloc.tensor_shape)
            dtype = mb.dt.np(alloc.dtype)
            out_names.append(name)
            out_avals.append(jax.core.ShapedArray(shape, dtype))
    n_params = len(in_names)
    all_names = in_names + out_names
    if partition_name is not None:
        all_names = all_names + [partition_name]

    def _body(*args):
        operands = list(args)
        if partition_name is not None:
            operands.append(partition_id_tensor())
        outs = _bass_exec_p.bind(
            *operands,
            out_avals=tuple(out_avals),
            in_names=tuple(all_names),
            out_names=tuple(out_names),
            lowering_input_output_aliases=(),
            sim_require_finite=True,
            sim_require_nnan=True,
            nc=nc,
        )
        return tuple(outs)

    devices = jax.devices()[:N_CORES]
    mesh = Mesh(np.asarray(devices), ("core",))
    n_outs = len(out_names)
    sharded = jax.jit(
        shard_map(
            _body, mesh=mesh,
            in_specs=(PartitionSpec("core"),) * (n_params + n_outs),
            out_specs=(PartitionSpec("core"),) * n_outs,
            check_rep=False,
        ),
        keep_unused=True,
    )
    zero_outs = [np.zeros((N_CORES * a.shape[0], *a.shape[1:]), a.dtype)
                 for a in out_avals]

    def run(in_maps):
        concat_in = [
            np.concatenate([np.asarray(in_maps[c][nm]) for c in range(N_CORES)], axis=0)
            for nm in in_names
        ]
        out_arrs = sharded(*concat_in, *zero_outs)
        out = np.asarray(out_arrs[0])
        return [out.reshape(N_CORES, *out_avals[0].shape)[c] for c in range(N_CORES)]

    _NC_CACHE[key] = (run, sharded, in_names, zero_outs)
    return _NC_CACHE[key]


def run_kernel_results(inputs, trace=False):
    run = _get_executor()[0]
    in_maps = _prep_inputs(inputs)
    outs = run(in_maps)
    full = np.concatenate(
        [r.reshape(NB, 256, 8, D) for r in outs], axis=0).astype(np.float32)
    return full, None


def kernel(**inputs):
    full, _ = run_kernel_results(inputs)
    return full
